# revision 10
# baseline (speedup 1.0000x reference)
"""Multi-head self-attention TRN2 Bass kernel.

Sharding: tensor-parallel over the 16 heads -> 2 heads per NeuronCore
(8 cores). Each core computes Q/K/V projections for its 128 head-dims
over all 4 batches, attention for its 8 (batch, head) pairs, and a
row-parallel slice of the output projection; the host sums the 8
partial outputs.

Layout trick: everything is kept transposed ([feature, token]) so the
PE contraction dim is always on partitions:
  qT/kT/vT  [128(2 heads x 64), 2048]   per batch
  dotsT     [128 keys, 512 queries]     = kT_tile.T @ qT_chunk
  P^T       = exp(dotsT/8)              (no max-subtraction needed; dots
                                         are bounded for this data)
  O'^T      [65, q] = V'.T @ P^T        accumulated over 16 key tiles,
                                         V' = [V * inv_k, inv_k] so row 64
                                         gives the softmax denominator and
                                         invalid keys are masked for free
  out       [tokens, 1024]              = (O^T/denom).T @ WcT, scaled by
                                         inv_t per token row (query mask)
Diagonal (self-attention) masking: multiply P^T tiles that contain the
diagonal by a precomputed (1-eye) pattern.

Matmuls run as float32r (full fp32 storage, relaxed-precision PE mode,
1 cycle/row for moving dim >= 256 vs 4 for plain fp32).
"""

import os
import numpy as np

import concourse.bacc as bacc
import concourse.bass as bass
import concourse.mybir as mybir
from concourse.tile import TileContext
from concourse.bass_utils import run_bass_kernel_spmd

B, S, H, NH, HD = 4, 2048, 1024, 16, 64
NCORES = 8
HPC = NH // NCORES        # heads per core = 2
PD = HPC * HD             # per-core projection dim = 128
FT = H // 128             # 8 feature k-tiles
KT = S // 128             # 16 key tiles of 128
QC = S // 512             # 4 query chunks of 512
F32 = mybir.dt.float32
F32R = mybir.dt.float32r

LAST_RESULTS = None       # BassKernelResults from the most recent run




def build_bass():
    nc = bacc.Bacc()
    xT = nc.dram_tensor("xT", [H, B * S], F32R, kind="ExternalInput")
    wT = {
        w: nc.dram_tensor(f"w{w}T", [H, PD], F32R, kind="ExternalInput")
        for w in "qkv"
    }
    bias = {
        w: nc.dram_tensor(f"b{w}", [PD, 1], F32, kind="ExternalInput")
        for w in "qkv"
    }
    wcT = nc.dram_tensor("wcT", [PD, H], F32R, kind="ExternalInput")
    inv2 = nc.dram_tensor("inv2", [128, B * KT], F32, kind="ExternalInput")
    diag = nc.dram_tensor("diag", [128, QC * 512], F32, kind="ExternalInput")
    iden = nc.dram_tensor("iden", [128, 64], F32R, kind="ExternalInput")
    ones = nc.dram_tensor("ones", [1, 64], F32, kind="ExternalInput")
    outp = nc.dram_tensor("out", [B * S, H], F32, kind="ExternalOutput")

    EXP = mybir.ActivationFunctionType.Exp
    IDENT = mybir.ActivationFunctionType.Identity

    with TileContext(nc) as tc, \
         tc.tile_pool(name="consts", bufs=1) as cpool, \
         tc.tile_pool(name="xt", bufs=10) as xpool, \
         tc.tile_pool(name="proj", bufs=1) as projpool, \
         tc.tile_pool(name="vp", bufs=2 * KT) as vppool, \
         tc.tile_pool(name="pt", bufs=4) as ptpool, \
         tc.tile_pool(name="onorm", bufs=2) as onpool, \
         tc.tile_pool(name="outsb", bufs=2) as outpool, \
         tc.tile_pool(name="small", bufs=2) as smallpool, \
         tc.tile_pool(name="psum", bufs=2, space="PSUM") as pspool:

        # ---- constants / weights (loaded once) ----
        w_sb = {}
        for w in "qkv":
            t = cpool.tile([128, FT * PD], F32R, name=f"w{w}sb")
            for ft in range(FT):
                nc.sync.dma_start(
                    out=t[:, ft * PD:(ft + 1) * PD],
                    in_=wT[w][ft * 128:(ft + 1) * 128, :])
            w_sb[w] = t
        b_sb = {}
        for w in "qkv":
            t = cpool.tile([128, 1], F32, name=f"b{w}sb")
            nc.sync.dma_start(out=t[:, :], in_=bias[w][:, :])
            b_sb[w] = t
        wc_sb = cpool.tile([128, H], F32R, name="wcsb")
        nc.sync.dma_start(out=wc_sb[:, :], in_=wcT[:, :])
        inv_sb = cpool.tile([128, B * KT], F32, name="invsb")
        nc.sync.dma_start(out=inv_sb[:, :], in_=inv2[:, :])
        diag_sb = cpool.tile([128, QC * 512], F32, name="diagsb")
        nc.sync.dma_start(out=diag_sb[:, :], in_=diag[:, :])
        iden_sb = cpool.tile([128, 64], F32R, name="idensb")
        nc.sync.dma_start(out=iden_sb[:, :], in_=iden[:, :])
        ones_sb = cpool.tile([1, 64], F32, name="onessb")
        nc.sync.dma_start(out=ones_sb[:, :], in_=ones[:, :])

        for b in range(B):
            tok0 = b * S
            # ---- Q/K/V projections -> qT/kT/vT [128, 2048] ----
            # x^T streamed in half-batches of 1024 tokens (8 tiles each)
            qkvT = {w: projpool.tile([128, S], F32R, tag=f"{w}T",
                                     name=f"{w}T{b}")
                    for w in "qkv"}
            for hb in range(2):
                hb0 = hb * 1024
                xt = []
                for ft in range(FT):
                    t = xpool.tile([128, 1024], F32R, tag="xt",
                                   name=f"xt{b}{hb}{ft}")
                    nc.sync.dma_start(
                        out=t[:, :],
                        in_=xT[ft * 128:(ft + 1) * 128,
                               tok0 + hb0:tok0 + hb0 + 1024])
                    xt.append(t)
                for w in "qkv":
                    for q2 in range(2):
                        pp = pspool.tile([128, 512], F32, tag="misc", bufs=2)
                        for ft in range(FT):
                            nc.tensor.matmul(
                                pp[:, :],
                                (w_sb[w][:, ft * PD:(ft + 1) * PD]),
                                (xt[ft][:, q2 * 512:(q2 + 1) * 512]),
                                start=(ft == 0), stop=(ft == FT - 1))
                        # PSUM -> SBUF with per-partition bias add
                        nc.scalar.activation(
                            qkvT[w][:, hb0 + q2 * 512:hb0 + (q2 + 1) * 512],
                            pp[:, :], IDENT, bias=b_sb[w][:, 0:1])

            # ---- V' build: [128 keys, 65] per (head, ktile) ----
            vp = {}
            for h in range(HPC):
                hsl = slice(h * HD, (h + 1) * HD)
                for kt in range(KT):
                    tp = pspool.tile([128, 64], F32R, tag="misc", bufs=2)
                    nc.tensor.transpose(
                        tp[:, :],
                        qkvT["v"][hsl, kt * 128:(kt + 1) * 128],
                        iden_sb[hsl, :])
                    vpt = vppool.tile([128, 72], F32R, tag="vp")
                    ic = inv_sb[:, b * KT + kt:b * KT + kt + 1]
                    nc.vector.tensor_scalar_mul(vpt[:, 0:64], tp[:, :], ic)
                    nc.vector.tensor_copy(vpt[:, 64:65], ic)
                    vp[(h, kt)] = vpt

            # ---- attention per head ----
            onorm = onpool.tile([128, S], F32R, tag="onorm")
            for h in range(HPC):
                hsl = slice(h * HD, (h + 1) * HD)
                avs = [pspool.tile([65, 512], F32, tag="av", bufs=4,
                                   name=f"av{b}{h}{qc}")
                       for qc in range(QC)]
                for kt in range(KT):
                    for qc in range(QC):
                        dp = pspool.tile([128, 512], F32, tag="dp", bufs=2)
                        nc.tensor.matmul(
                            dp[:, :],
                            (qkvT["k"][hsl, kt * 128:(kt + 1) * 128]),
                            (qkvT["q"][hsl, qc * 512:(qc + 1) * 512]),
                            start=True, stop=True)
                        pt = ptpool.tile([128, 512], F32R, tag="pt")
                        nc.scalar.activation(pt[:, :], dp[:, :], EXP,
                                             scale=0.125)
                        if kt // 4 == qc:
                            j = kt % 4
                            nc.vector.tensor_mul(
                                pt[:, :], pt[:, :],
                                diag_sb[:, j * 512:(j + 1) * 512])
                        nc.tensor.matmul(
                            avs[qc][:, :],
                            (vp[(h, kt)][:, 0:65]),
                            (pt[:, :]),
                            start=(kt == 0), stop=(kt == KT - 1))
                # normalize: onorm[h] = O_unnorm / denom
                for qc in range(QC):
                    rc = smallpool.tile([1, 512], F32, tag="rc")
                    den = smallpool.tile([1, 512], F32, tag="den")
                    nc.vector.tensor_scalar_max(
                        den[:, :], avs[qc][64:65, :], 1e-30)
                    nc.vector.reciprocal(rc[:, :], den[:, :])
                    rep = pspool.tile([64, 512], F32, tag="misc", bufs=2)
                    nc.tensor.matmul(rep[:, :], ones_sb[:, :], rc[:, :],
                                     start=True, stop=True)
                    rep_sb = smallpool.tile([64, 512], F32, tag="repsb")
                    nc.scalar.copy(rep_sb[:, :], rep[:, :])
                    nc.vector.tensor_mul(
                        onorm[hsl, qc * 512:(qc + 1) * 512],
                        avs[qc][0:64, :], rep_sb[:, :])

            # ---- output projection + query-mask scaling ----
            for tt in range(KT):
                osb = outpool.tile([128, H], F32, tag="osb")
                for oc in range(2):
                    op = pspool.tile([128, 512], F32, tag="misc", bufs=2)
                    nc.tensor.matmul(
                        op[:, :],
                        onorm[:, tt * 128:(tt + 1) * 128],
                        wc_sb[:, oc * 512:(oc + 1) * 512],
                        start=True, stop=True)
                    nc.vector.tensor_scalar_mul(
                        osb[:, oc * 512:(oc + 1) * 512], op[:, :],
                        inv_sb[:, b * KT + tt:b * KT + tt + 1])
                nc.sync.dma_start(
                    out=outp[tok0 + tt * 128:tok0 + (tt + 1) * 128, :],
                    in_=osb[:, :])
    nc.finalize()
    return nc


_NC_CACHE = None


def kernel(encoder_outputs, mask, Wq, bq, Wk, bk, Wv, bv, Wc):
    global LAST_RESULTS, _NC_CACHE
    x = np.asarray(encoder_outputs, dtype=np.float32)
    xT = np.ascontiguousarray(x.reshape(B * S, H).T)
    inv = (1.0 - np.asarray(mask)).astype(np.float32)            # [B, S]
    inv2 = np.ascontiguousarray(
        inv.reshape(B, KT, 128).transpose(2, 0, 1).reshape(128, B * KT))
    diagpat = np.ones((128, QC * 512), dtype=np.float32)
    for j in range(QC):
        idx = np.arange(128)
        diagpat[idx, j * 512 + j * 128 + idx] = 0.0
    iden = np.zeros((128, 64), dtype=np.float32)
    iden[0:64] = np.eye(64, dtype=np.float32)
    iden[64:128] = np.eye(64, dtype=np.float32)
    onesv = np.ones((1, 64), dtype=np.float32)

    in_maps = []
    for c in range(NCORES):
        sl = slice(c * PD, (c + 1) * PD)
        in_maps.append({
            "xT": xT,
            "wqT": np.ascontiguousarray(np.asarray(Wq, np.float32)[sl, :].T),
            "wkT": np.ascontiguousarray(np.asarray(Wk, np.float32)[sl, :].T),
            "wvT": np.ascontiguousarray(np.asarray(Wv, np.float32)[sl, :].T),
            "bq": np.asarray(bq, np.float32)[sl].reshape(PD, 1).copy(),
            "bk": np.asarray(bk, np.float32)[sl].reshape(PD, 1).copy(),
            "bv": np.asarray(bv, np.float32)[sl].reshape(PD, 1).copy(),
            "wcT": np.ascontiguousarray(np.asarray(Wc, np.float32)[:, sl].T),
            "inv2": inv2,
            "diag": diagpat,
            "iden": iden,
            "ones": onesv,
        })

    if _NC_CACHE is None:
        _NC_CACHE = build_bass()
    res = run_bass_kernel_spmd(
        _NC_CACHE, in_maps, list(range(NCORES)),
        trace=bool(os.environ.get("BASS_TRACE")))
    LAST_RESULTS = res
    out = np.zeros((B * S, H), dtype=np.float32)
    for c in range(NCORES):
        out += res.results[c]["out"]
    return out.reshape(B, S, H)
